# revision 12
# baseline (speedup 1.0000x reference)
"""Trainium2 Bass kernel for CausalSelfAttention with sliding-window + sink mask.

Sharding: 8 cores = (batch 2) x (sequence chunks of 512). Each core computes
QKV (+RoPE) for its 512 queries and a tight kv range [512 own | 256 halo |
4 sink] = 772 positions, runs banded attention in a scores-transposed [k, q]
layout with chunks packed into two [128, 1024] PSUM supertiles per head-half
(multiplicative post-exp masking, denominator via a ones-column in V), then
projects with w_proj emitting a transposed [C, 512] output that the host
re-transposes and concatenates.

v2: bf16 operands everywhere (weights, x, attention internals; fp32 PSUM),
packed score tiles to halve exp/mask instruction count, engine
load-balancing (exp on scalar, rope/mask/norm on vector, copies on gpsimd),
and an interleaved PE schedule that keeps the tensor engine dense so the
HAM clock stays at 2.4 GHz.
"""

import numpy as np

B, T, C, NH, HD = 2, 2048, 1024, 16, 64
WIN, SINK = 256, 4
CH = 512          # queries per core
KV = 772          # 512 own + 256 halo + 4 sink
KVP = 896         # padded key space for 7x128 score chunking
NCORES = 8

# Packed score-tile layout: (tile_idx, col_off, key_chunk, W, q_off).
# Key chunks: 0-3 own kv[0:512], 4 halo-lo kv[512:640], 5 halo-hi
# kv[640:768], 6 sink kv[768:772]+pad. Each supertile is [128, 1024]
# (2 PSUM banks); no matmul write crosses a 512-col bank boundary.
CHUNKS = [
    (0, 0,   0, 384, 0),
    (0, 384, 3, 128, 384),
    (0, 512, 1, 384, 128),
    (0, 896, 4, 128, 0),
    (1, 0,   2, 256, 256),
    (1, 256, 5, 256, 0),
    (1, 512, 6, 512, 0),
]
MTOT = 2048

_cache = {}


def _build_nc():
    import concourse.bacc as bacc
    import concourse.mybir as mybir
    import concourse.tile as tile

    f32 = mybir.dt.float32
    bf16 = mybir.dt.bfloat16
    AF = mybir.ActivationFunctionType

    nc = bacc.Bacc("TRN2", target_bir_lowering=False, debug=False,
                   num_devices=NCORES)

    xTd = nc.dram_tensor("xT", [128, 8 * KV], bf16, kind="ExternalInput").ap()
    wqd = nc.dram_tensor("wqs", [128, 8 * C], bf16, kind="ExternalInput").ap()
    wkd = nc.dram_tensor("wks", [128, 8 * C], bf16, kind="ExternalInput").ap()
    wvd = nc.dram_tensor("wv", [128, 8 * C], bf16, kind="ExternalInput").ap()
    wpd = nc.dram_tensor("wps", [128, 8 * C], bf16, kind="ExternalInput").ap()
    cqd = nc.dram_tensor("cos_q", [128, CH], bf16, kind="ExternalInput").ap()
    sqd = nc.dram_tensor("sin_q", [128, CH], bf16, kind="ExternalInput").ap()
    ckd = nc.dram_tensor("cos_k", [128, KV], bf16, kind="ExternalInput").ap()
    skd = nc.dram_tensor("sin_k", [128, KV], bf16, kind="ExternalInput").ap()
    maskd = nc.dram_tensor("masks", [128, MTOT], bf16, kind="ExternalInput").ap()
    p2d = nc.dram_tensor("p2", [128, 128], bf16, kind="ExternalInput").ap()
    sel2d = nc.dram_tensor("sel2", [2, 128], bf16, kind="ExternalInput").ap()
    onesd = nc.dram_tensor("ones", [128, 16], bf16, kind="ExternalInput").ap()
    outT = nc.dram_tensor("outT", [C, CH], bf16, kind="ExternalOutput").ap()

    with tile.TileContext(nc) as tc:
        with (
            tc.tile_pool(name="pers", bufs=1) as pers,
            tc.tile_pool(name="tmp", bufs=2) as tmp,
            tc.tile_pool(name="tmp2", bufs=2) as tmp2,
            tc.tile_pool(name="qk", bufs=2) as qkp,
            tc.tile_pool(name="ptp", bufs=8) as ptp,
            tc.tile_pool(name="big", bufs=2) as big,
            tc.tile_pool(name="psk", bufs=1, space="PSUM") as psk,
            tc.tile_pool(name="pssc", bufs=2, space="PSUM") as pssc,
            tc.tile_pool(name="psmm", bufs=2, space="PSUM") as psmm,
        ):
            # ---------- persistent loads (packed lines, priority order) ---
            xall = pers.tile([128, 8 * KV], bf16, tag="xall")
            nc.sync.dma_start(xall[:], xTd[:])
            xab = [xall[:, i * KV:(i + 1) * KV] for i in range(8)]

            wqall = pers.tile([128, 8 * C], bf16, tag="wqall")
            wkall = pers.tile([128, 8 * C], bf16, tag="wkall")
            wvall = pers.tile([128, 8 * C], bf16, tag="wvall")
            wpall = pers.tile([128, 8 * C], bf16, tag="wpall")
            wq_t = [wqall[:, i * C:(i + 1) * C] for i in range(8)]
            wk_t = [wkall[:, i * C:(i + 1) * C] for i in range(8)]
            wv_t = [wvall[:, i * C:(i + 1) * C] for i in range(8)]
            wp_t = [wpall[:, i * C:(i + 1) * C] for i in range(8)]

            nc.sync.dma_start(wqall[:, 0:C], wqd[:, 0:C])
            nc.sync.dma_start(wkall[:, 0:C], wkd[:, 0:C])

            tp2 = pers.tile([128, 128], bf16, tag="p2")
            nc.sync.dma_start(tp2[:], p2d[:])
            tcos_q = pers.tile([128, CH], bf16, tag="cos_q")
            nc.sync.dma_start(tcos_q[:], cqd[:])
            tsin_q = pers.tile([128, CH], bf16, tag="sin_q")
            nc.sync.dma_start(tsin_q[:], sqd[:])
            tcos_k = pers.tile([128, KV], bf16, tag="cos_k")
            nc.sync.dma_start(tcos_k[:], ckd[:])
            tsin_k = pers.tile([128, KV], bf16, tag="sin_k")
            nc.sync.dma_start(tsin_k[:], skd[:])

            nc.sync.dma_start(wvall[:], wvd[:])
            tmask = pers.tile([128, MTOT], bf16, tag="mask")
            nc.sync.dma_start(tmask[:], maskd[:])
            tones = pers.tile([128, 16], bf16, tag="ones")
            nc.sync.dma_start(tones[:], onesd[:])
            nc.sync.dma_start(wqall[:, C:8 * C], wqd[:, C:8 * C])
            nc.sync.dma_start(wkall[:, C:8 * C], wkd[:, C:8 * C])
            tsel2 = pers.tile([2, 128], bf16, tag="sel2")
            nc.sync.dma_start(tsel2[:], sel2d[:])
            nc.sync.dma_start(wpall[:], wpd[:])

            ytu = [pers.tile([128, CH], bf16, tag=f"ytu{i}", name=f"ytu{i}")
                   for i in range(8)]

            # ---------- rope/QKV halves (interleaved into the hp loop) ----
            def rope_q(hp):
                pq = psmm.tile([128, CH], f32, tag="mm", name=f"pq{hp}")
                for kc in range(8):
                    nc.tensor.matmul(
                        pq[:], wq_t[hp][:, kc * 128:(kc + 1) * 128],
                        xab[kc][:, 0:CH], start=(kc == 0), stop=(kc == 7),
                    )
                qraw = tmp.tile([128, CH], bf16, tag="qraw", name=f"qraw{hp}")
                nc.scalar.copy(qraw[:], pq[:])
                prot = psmm.tile([128, CH], f32, tag="mm", name=f"prot{hp}")
                nc.tensor.matmul(prot[:], tp2[:], qraw[:], start=True,
                                 stop=True)
                t2 = tmp2.tile([128, KV], bf16, tag="t2", name=f"t2q{hp}")
                nc.vector.tensor_mul(t2[:, 0:CH], prot[:], tsin_q[:])
                qc = tmp2.tile([128, KV], bf16, tag="tc", name=f"qc{hp}")
                nc.gpsimd.tensor_mul(qc[:, 0:CH], qraw[:], tcos_q[:])
                qT = qkp.tile([128, CH], bf16, tag="qT", name=f"qT{hp}")
                nc.vector.tensor_add(qT[:], qc[:, 0:CH], t2[:, 0:CH])
                return qT

            def rope_k(hp):
                pk = psk.tile([128, KV], f32, tag="kk", name=f"pk{hp}")
                for kc in range(8):
                    nc.tensor.matmul(
                        pk[:, 0:512], wk_t[hp][:, kc * 128:(kc + 1) * 128],
                        xab[kc][:, 0:512], start=(kc == 0), stop=(kc == 7),
                    )
                for kc in range(8):
                    nc.tensor.matmul(
                        pk[:, 512:KV], wk_t[hp][:, kc * 128:(kc + 1) * 128],
                        xab[kc][:, 512:KV], start=(kc == 0), stop=(kc == 7),
                    )
                kraw = tmp.tile([128, KV], bf16, tag="kraw", name=f"kraw{hp}")
                nc.vector.tensor_copy(kraw[:], pk[:])
                prk = psk.tile([128, KV], f32, tag="kk", name=f"prk{hp}")
                nc.tensor.matmul(prk[:, 0:512], tp2[:], kraw[:, 0:512],
                                 start=True, stop=True)
                nc.tensor.matmul(prk[:, 512:KV], tp2[:], kraw[:, 512:KV],
                                 start=True, stop=True)
                t2k = tmp2.tile([128, KV], bf16, tag="t2", name=f"t2k{hp}")
                nc.vector.tensor_mul(t2k[:], prk[:], tsin_k[:])
                kc_ = tmp2.tile([128, KV], bf16, tag="tc", name=f"kc{hp}")
                nc.gpsimd.tensor_mul(kc_[:], kraw[:], tcos_k[:])
                kT = qkp.tile([128, KVP], bf16, tag="kT", name=f"kT{hp}")
                nc.gpsimd.memset(kT[:, KV:KVP], 0.0)
                nc.vector.tensor_add(kT[:, 0:KV], kc_[:], t2k[:])
                return kT

            # ---------- V = xT.T @ wv in [k, d] layout with ones column ----
            qT0 = rope_q(0)
            kT0 = rope_k(0)

            v_sb = []
            for tt in range(7):
                vt = pers.tile([128, 1040], bf16, tag=f"v{tt}", name=f"v{tt}")
                vr = vt.rearrange("p (h e) -> p h e", e=65)
                if tt == 6:
                    nc.gpsimd.memset(vt[:], 0.0)
                pv = [psmm.tile([128, CH], f32, tag="mm", name=f"pv{tt}_{i}")
                      for i in range(2)]
                for kc in range(8):
                    if tt == 6:
                        xsl = xab[kc][:, 768:772]
                    else:
                        xsl = xab[kc][:, tt * 128:(tt + 1) * 128]
                    for dh in range(2):
                        nc.tensor.matmul(
                            pv[dh][:, 0:CH] if tt != 6 else pv[dh][0:4, 0:CH],
                            xsl, wv_t[kc][:, dh * 512:(dh + 1) * 512],
                            start=(kc == 0), stop=(kc == 7),
                        )
                nrow = 128 if tt != 6 else 4
                for dh in range(2):
                    nc.scalar.copy(
                        vr[0:nrow, dh * 8:(dh + 1) * 8, 0:64],
                        pv[dh][0:nrow].rearrange("p (h e) -> p h e", e=64),
                    )
                nc.scalar.copy(vr[:, :, 64:65],
                               tones[:].rearrange("p (h o) -> p h o", o=1))
                v_sb.append(vt)

            # ---------- attention blocks ----------
            def sc_half(hp, half, qT, kT):
                dsl = slice(half * 64, half * 64 + 64)
                sts = []
                for tidx in range(2):
                    st = pssc.tile([128, 1024], f32, tag="sc",
                                   name=f"st{hp}_{half}_{tidx}")
                    for (ti, coff, kc, w, qoff) in CHUNKS:
                        if ti != tidx:
                            continue
                        nc.tensor.matmul(
                            st[:, coff:coff + w],
                            kT[dsl, kc * 128:(kc + 1) * 128],
                            qT[dsl, qoff:qoff + w], start=True, stop=True,
                        )
                    sts.append(st)
                return sts

            def exp_mask(hp, half, sts):
                pts = []
                for tidx, st in enumerate(sts):
                    praw = ptp.tile([128, 1024], bf16, tag="pt",
                                    name=f"praw{hp}_{half}_{tidx}")
                    nc.scalar.activation(praw[:], st[:], AF.Exp, scale=0.125)
                    pt = ptp.tile([128, 1024], bf16, tag="pt",
                                  name=f"pt{hp}_{half}_{tidx}")
                    nc.vector.tensor_mul(
                        pt[:], praw[:],
                        tmask[:, tidx * 1024:(tidx + 1) * 1024])
                    pts.append(pt)
                return pts

            # AV order: sink chunk first — its full 512-col window
            # initializes every yt element (start=True), the rest accumulate.
            AV_CHUNKS = [CHUNKS[6]] + CHUNKS[0:6]

            def av_half(hp, half, pts, d2):
                h = hp * 2 + half
                yt = psmm.tile([128, CH], f32, tag="mm",
                               name=f"yt{hp}_{half}")
                for ci, (ti, coff, kc, w, qoff) in enumerate(AV_CHUNKS):
                    nc.tensor.matmul(
                        yt[0:65, qoff:qoff + w],
                        v_sb[kc][:, h * 65:(h + 1) * 65],
                        pts[ti][:, coff:coff + w],
                        start=(ci == 0), stop=(ci == 6),
                        skip_group_check=True,
                    )
                # stage denominator + numerator out of PSUM (engines cannot
                # write at a partition offset, so bounce the denom via DMA)
                dt_ = tmp.tile([1, CH], f32, tag="dt", name=f"dt{hp}_{half}")
                nc.scalar.copy(dt_[:], yt[64:65, :])
                nc.sync.dma_start(d2[half:half + 1, :], dt_[:])
                nc.scalar.copy(ytu[hp][half * 64:half * 64 + 64, :],
                               yt[0:64, :])

            def norm_hp(hp, d2):
                # per-head-pair normalization: 1/denominator broadcast down
                # 64 partitions via a tiny matmul, then one fused multiply
                r2f = tmp.tile([2, CH], f32, tag="r2f", name=f"r2f{hp}")
                nc.vector.reciprocal_approx_fast(r2f[:], d2[:])
                r2 = tmp.tile([2, CH], bf16, tag="r2", name=f"r2{hp}")
                nc.vector.tensor_copy(r2[:], r2f[:])
                prb = psmm.tile([128, CH], f32, tag="mm", name=f"prb{hp}")
                nc.tensor.matmul(prb[:], tsel2[:], r2[:], start=True,
                                 stop=True)
                nc.vector.tensor_mul(ytu[hp][:], ytu[hp][:], prb[:])

            # ---------- software-pipelined head-pair loop ----------
            qT, kT = qT0, kT0
            for hp in range(8):
                sts0 = sc_half(hp, 0, qT, kT)
                pts0 = exp_mask(hp, 0, sts0)
                nqT = rope_q(hp + 1) if hp < 7 else None
                sts1 = sc_half(hp, 1, qT, kT)
                pts1 = exp_mask(hp, 1, sts1)
                nkT = rope_k(hp + 1) if hp < 7 else None
                d2 = tmp.tile([2, CH], f32, tag="d2", name=f"d2_{hp}")
                av_half(hp, 0, pts0, d2)
                av_half(hp, 1, pts1, d2)
                norm_hp(hp, d2)
                qT, kT = nqT, nkT

            # ---------- projection (transposed output) ----------
            for cc in range(8):
                po = psmm.tile([128, CH], f32, tag="mm", name=f"po{cc}")
                for hp in range(8):
                    nc.tensor.matmul(
                        po[:], wp_t[cc][:, hp * 128:(hp + 1) * 128],
                        ytu[hp][:], start=(hp == 0), stop=(hp == 7),
                    )
                osb = big.tile([128, CH], bf16, tag="osb", name=f"osb{cc}")
                nc.scalar.copy(osb[:], po[:])
                nc.sync.dma_start(outT[cc * 128:(cc + 1) * 128, :], osb[:])

    nc.compile()
    return nc


def _host_inputs(x, w_attn, w_proj):
    """Build the 8 per-core input maps (bf16 operands)."""
    import ml_dtypes
    bf = ml_dtypes.bfloat16

    inv_freq = 1.0 / (10000.0 ** (np.arange(0, HD, 2, dtype=np.float32) / HD))
    iff = np.concatenate([inv_freq, inv_freq])  # [64]

    def cos_sin(pos):
        ang = pos[None, :].astype(np.float32) * iff[:, None]
        c = np.concatenate([np.cos(ang), np.cos(ang)], 0)
        s = np.concatenate([np.sin(ang), np.sin(ang)], 0)
        return (np.ascontiguousarray(c.astype(bf)),
                np.ascontiguousarray(s.astype(bf)))

    P2 = np.zeros((128, 128), np.float32)
    for blk in range(2):
        o = blk * 64
        for d in range(32):
            P2[o + d + 32, o + d] = -1.0
            P2[o + d, o + d + 32] = 1.0

    sel2 = np.zeros((2, 128), np.float32)
    sel2[0, 0:64] = 1.0
    sel2[1, 64:128] = 1.0
    ones16 = np.ones((128, 16), np.float32)

    def shuffle_lhsT(w):
        # rows (kc*128 + c_lo), cols (hp*128 + d) ->
        # rows (hp*128 + c_lo), cols (kc*128 + d), then packed so SBUF
        # partition p carries all 8 hp-blocks contiguously (16KB DMA lines)
        sh = w.reshape(8, 128, 8, 128).transpose(2, 1, 0, 3).reshape(C, C)
        return np.ascontiguousarray(
            sh.reshape(8, 128, C).transpose(1, 0, 2).reshape(128, 8 * C))

    def pack_rows(w):
        # [8*128, N] -> [128, 8*N]: partition p holds all 8 row-blocks
        n = w.shape[1]
        return np.ascontiguousarray(
            w.reshape(8, 128, n).transpose(1, 0, 2).reshape(128, 8 * n))

    wq = shuffle_lhsT(w_attn[:, 0:C]).astype(bf)
    wk = shuffle_lhsT(w_attn[:, C:2 * C]).astype(bf)
    wvm = pack_rows(np.ascontiguousarray(w_attn[:, 2 * C:3 * C])).astype(bf)
    wp = shuffle_lhsT(w_proj).astype(bf)
    P2 = P2.astype(bf)
    sel2 = sel2.astype(bf)
    ones16 = ones16.astype(bf)

    in_maps = []
    for core in range(NCORES):
        b, j = core // 4, core % 4
        q0 = j * CH
        # kv layout: [512 own | 256 halo | 4 sink]
        kv_gk = np.full(KV, -1, np.int64)
        kv_gk[0:512] = q0 + np.arange(CH)
        halo = q0 - 256 + np.arange(256)
        kv_gk[512:768] = np.where(halo >= 0, halo, -1)
        kv_gk[768:772] = np.arange(4)

        xTc = np.zeros((C, KV), np.float32)
        valid = kv_gk >= 0
        xTc[:, valid] = x[b, kv_gk[valid]].T

        cq, sq = cos_sin(q0 + np.arange(CH))
        ck, sk = cos_sin(np.maximum(kv_gk, 0))

        mask = np.zeros((128, MTOT), np.float32)
        for (tidx, coff, kc, w, qoff) in CHUNKS:
            p = np.arange(128)
            if kc < 4:
                g = q0 + kc * 128 + p
                real = np.ones(128, bool)
            elif kc in (4, 5):
                g = q0 - 256 + (kc - 4) * 128 + p
                real = g >= 0
            else:
                g = p.copy()
                real = p < 4
            gq = q0 + qoff + np.arange(w)
            gcol = np.where(real, g, 0)[:, None]
            qq = gq[None, :]
            if kc == 6:
                allow = (gcol <= qq) & (qq - gcol >= WIN)
            else:
                allow = (gcol <= qq) & (qq - gcol < WIN)
            allow &= real[:, None]
            mask[:, tidx * 1024 + coff: tidx * 1024 + coff + w] = \
                allow.astype(np.float32)

        in_maps.append({
            "xT": pack_rows(xTc).astype(bf), "wqs": wq, "wks": wk,
            "wv": wvm, "wps": wp,
            "cos_q": cq, "sin_q": sq, "cos_k": ck, "sin_k": sk,
            "masks": mask.astype(bf), "p2": P2, "sel2": sel2, "ones": ones16,
        })
    return in_maps


def kernel(x, w_attn, w_proj):
    from concourse import bass_utils

    x = np.asarray(x, np.float32)
    w_attn = np.asarray(w_attn, np.float32)
    w_proj = np.asarray(w_proj, np.float32)

    if "nc" not in _cache:
        _cache["nc"] = _build_nc()
    nc = _cache["nc"]

    in_maps = _host_inputs(x, w_attn, w_proj)
    res = bass_utils.run_bass_kernel_spmd(nc, in_maps, list(range(NCORES)),
                                          **_cache.get("run_kwargs", {}))
    _cache["last_result"] = res

    y = np.zeros((B, T, C), np.float32)
    for core in range(NCORES):
        b, j = core // 4, core % 4
        y[b, j * CH:(j + 1) * CH, :] = \
            np.asarray(res.results[core]["outT"], np.float32).T
    return y


# revision 14
# speedup vs baseline: 1.0423x; 1.0423x over previous
"""Trainium2 Bass kernel for CausalSelfAttention with sliding-window + sink mask.

Sharding: 8 cores = (batch 2) x (sequence chunks of 512). Each core computes
QKV (+RoPE) for its 512 queries and a tight kv range [512 own | 256 halo |
4 sink] = 772 positions, runs banded attention in a scores-transposed [k, q]
layout with chunks packed into two [128, 1024] PSUM supertiles per head-half
(multiplicative post-exp masking, denominator via a ones-column in V), then
projects with w_proj emitting a transposed [C, 512] output that the host
re-transposes and concatenates.

v2: bf16 operands everywhere (weights, x, attention internals; fp32 PSUM),
packed score tiles to halve exp/mask instruction count, engine
load-balancing (exp on scalar, rope/mask/norm on vector, copies on gpsimd),
and an interleaved PE schedule that keeps the tensor engine dense so the
HAM clock stays at 2.4 GHz.
"""

import numpy as np

B, T, C, NH, HD = 2, 2048, 1024, 16, 64
WIN, SINK = 256, 4
CH = 512          # queries per core
KV = 772          # 512 own + 256 halo + 4 sink
KVP = 896         # padded key space for 7x128 score chunking
NCORES = 8

# Packed score-tile layout: (tile_idx, col_off, key_chunk, W, q_off).
# Key chunks: 0-3 own kv[0:512], 4 halo-lo kv[512:640], 5 halo-hi
# kv[640:768], 6 sink kv[768:772]+pad. Each supertile is [128, 1024]
# (2 PSUM banks); no matmul write crosses a 512-col bank boundary.
CHUNKS = [
    (0, 0,   0, 384, 0),
    (0, 384, 3, 128, 384),
    (0, 512, 1, 384, 128),
    (0, 896, 4, 128, 0),
    (1, 0,   2, 256, 256),
    (1, 256, 5, 256, 0),
    (1, 512, 6, 512, 0),
]
MTOT = 2048

_cache = {}


def _build_nc():
    import concourse.bacc as bacc
    import concourse.mybir as mybir
    import concourse.tile as tile

    f32 = mybir.dt.float32
    bf16 = mybir.dt.bfloat16
    AF = mybir.ActivationFunctionType

    nc = bacc.Bacc("TRN2", target_bir_lowering=False, debug=False,
                   num_devices=NCORES)

    xTd = nc.dram_tensor("xT", [128, 8 * KV], bf16, kind="ExternalInput").ap()
    wqd = nc.dram_tensor("wqs", [128, 8 * C], bf16, kind="ExternalInput").ap()
    wkd = nc.dram_tensor("wks", [128, 8 * C], bf16, kind="ExternalInput").ap()
    wvd = nc.dram_tensor("wv", [128, 8 * C], bf16, kind="ExternalInput").ap()
    wpd = nc.dram_tensor("wps", [128, 8 * C], bf16, kind="ExternalInput").ap()
    cqd = nc.dram_tensor("cos_q", [128, CH], bf16, kind="ExternalInput").ap()
    sqd = nc.dram_tensor("sin_q", [128, CH], bf16, kind="ExternalInput").ap()
    ckd = nc.dram_tensor("cos_k", [128, KV], bf16, kind="ExternalInput").ap()
    skd = nc.dram_tensor("sin_k", [128, KV], bf16, kind="ExternalInput").ap()
    maskd = nc.dram_tensor("masks", [128, MTOT], bf16, kind="ExternalInput").ap()
    p2d = nc.dram_tensor("p2", [128, 128], bf16, kind="ExternalInput").ap()
    sel2d = nc.dram_tensor("sel2", [2, 128], bf16, kind="ExternalInput").ap()
    onesd = nc.dram_tensor("ones", [128, 16], bf16, kind="ExternalInput").ap()
    outT = nc.dram_tensor("outT", [128, 8 * CH], bf16,
                          kind="ExternalOutput").ap()

    with tile.TileContext(nc) as tc:
        with (
            tc.tile_pool(name="pers", bufs=1) as pers,
            tc.tile_pool(name="tmp", bufs=2) as tmp,
            tc.tile_pool(name="tmp2", bufs=2) as tmp2,
            tc.tile_pool(name="qk", bufs=2) as qkp,
            tc.tile_pool(name="ptp", bufs=8) as ptp,
            tc.tile_pool(name="big", bufs=1) as big,
            tc.tile_pool(name="psk", bufs=1, space="PSUM") as psk,
            tc.tile_pool(name="pssc", bufs=2, space="PSUM") as pssc,
            tc.tile_pool(name="psmm", bufs=2, space="PSUM") as psmm,
        ):
            # ---------- persistent loads (packed lines, priority order) ---
            xall = pers.tile([128, 8 * KV], bf16, tag="xall")
            nc.sync.dma_start(xall[:], xTd[:])
            xab = [xall[:, i * KV:(i + 1) * KV] for i in range(8)]

            wqall = pers.tile([128, 8 * C], bf16, tag="wqall")
            wkall = pers.tile([128, 8 * C], bf16, tag="wkall")
            wvall = pers.tile([128, 8 * C], bf16, tag="wvall")
            wpall = pers.tile([128, 8 * C], bf16, tag="wpall")
            wq_t = [wqall[:, i * C:(i + 1) * C] for i in range(8)]
            wk_t = [wkall[:, i * C:(i + 1) * C] for i in range(8)]
            wv_t = [wvall[:, i * C:(i + 1) * C] for i in range(8)]
            wp_t = [wpall[:, i * C:(i + 1) * C] for i in range(8)]

            nc.sync.dma_start(wqall[:, 0:C], wqd[:, 0:C])
            nc.sync.dma_start(wkall[:, 0:C], wkd[:, 0:C])

            tp2 = pers.tile([128, 128], bf16, tag="p2")
            nc.sync.dma_start(tp2[:], p2d[:])
            tcos_q = pers.tile([128, CH], bf16, tag="cos_q")
            nc.sync.dma_start(tcos_q[:], cqd[:])
            tsin_q = pers.tile([128, CH], bf16, tag="sin_q")
            nc.sync.dma_start(tsin_q[:], sqd[:])

            nc.sync.dma_start(wvall[:], wvd[:])

            tcos_k = pers.tile([128, KV], bf16, tag="cos_k")
            nc.sync.dma_start(tcos_k[:], ckd[:])
            tsin_k = pers.tile([128, KV], bf16, tag="sin_k")
            nc.sync.dma_start(tsin_k[:], skd[:])
            tmask = pers.tile([128, MTOT], bf16, tag="mask")
            nc.sync.dma_start(tmask[:], maskd[:])
            tones = pers.tile([128, 16], bf16, tag="ones")
            nc.sync.dma_start(tones[:], onesd[:])
            nc.sync.dma_start(wqall[:, C:8 * C], wqd[:, C:8 * C])
            nc.sync.dma_start(wkall[:, C:8 * C], wkd[:, C:8 * C])
            tsel2 = pers.tile([2, 128], bf16, tag="sel2")
            nc.sync.dma_start(tsel2[:], sel2d[:])
            nc.sync.dma_start(wpall[:], wpd[:])

            ytu = [pers.tile([128, CH], bf16, tag=f"ytu{i}", name=f"ytu{i}")
                   for i in range(8)]

            # ---------- rope/QKV halves (interleaved into the hp loop) ----
            def rope_q(hp):
                pq = psmm.tile([128, CH], f32, tag="mm", name=f"pq{hp}")
                for kc in range(8):
                    nc.tensor.matmul(
                        pq[:], wq_t[hp][:, kc * 128:(kc + 1) * 128],
                        xab[kc][:, 0:CH], start=(kc == 0), stop=(kc == 7),
                    )
                qraw = tmp.tile([128, CH], bf16, tag="qraw", name=f"qraw{hp}")
                nc.scalar.copy(qraw[:], pq[:])
                prot = psmm.tile([128, CH], f32, tag="mm", name=f"prot{hp}")
                nc.tensor.matmul(prot[:], tp2[:], qraw[:], start=True,
                                 stop=True)
                t2 = tmp2.tile([128, KV], bf16, tag="t2", name=f"t2q{hp}")
                nc.vector.tensor_mul(t2[:, 0:CH], prot[:], tsin_q[:])
                qc = tmp2.tile([128, KV], bf16, tag="tc", name=f"qc{hp}")
                nc.gpsimd.tensor_mul(qc[:, 0:CH], qraw[:], tcos_q[:])
                qT = qkp.tile([128, CH], bf16, tag="qT", name=f"qT{hp}")
                nc.vector.tensor_add(qT[:], qc[:, 0:CH], t2[:, 0:CH])
                return qT

            def rope_k(hp):
                pk = psk.tile([128, KV], f32, tag="kk", name=f"pk{hp}")
                for kc in range(8):
                    nc.tensor.matmul(
                        pk[:, 0:512], wk_t[hp][:, kc * 128:(kc + 1) * 128],
                        xab[kc][:, 0:512], start=(kc == 0), stop=(kc == 7),
                    )
                for kc in range(8):
                    nc.tensor.matmul(
                        pk[:, 512:KV], wk_t[hp][:, kc * 128:(kc + 1) * 128],
                        xab[kc][:, 512:KV], start=(kc == 0), stop=(kc == 7),
                    )
                kraw = tmp.tile([128, KV], bf16, tag="kraw", name=f"kraw{hp}")
                nc.vector.tensor_copy(kraw[:], pk[:])
                prk = psk.tile([128, KV], f32, tag="kk", name=f"prk{hp}")
                nc.tensor.matmul(prk[:, 0:512], tp2[:], kraw[:, 0:512],
                                 start=True, stop=True)
                nc.tensor.matmul(prk[:, 512:KV], tp2[:], kraw[:, 512:KV],
                                 start=True, stop=True)
                t2k = tmp2.tile([128, KV], bf16, tag="t2", name=f"t2k{hp}")
                nc.vector.tensor_mul(t2k[:], prk[:], tsin_k[:])
                kc_ = tmp2.tile([128, KV], bf16, tag="tc", name=f"kc{hp}")
                nc.gpsimd.tensor_mul(kc_[:], kraw[:], tcos_k[:])
                kT = qkp.tile([128, KVP], bf16, tag="kT", name=f"kT{hp}")
                nc.gpsimd.memset(kT[:, KV:KVP], 0.0)
                nc.vector.tensor_add(kT[:, 0:KV], kc_[:], t2k[:])
                return kT

            # ---------- V = xT.T @ wv in [k, d] layout with ones column ----
            qT0 = rope_q(0)
            kT0 = rope_k(0)

            v_sb = []
            for tt in range(7):
                vt = pers.tile([128, 1040], bf16, tag=f"v{tt}", name=f"v{tt}")
                vr = vt.rearrange("p (h e) -> p h e", e=65)
                if tt == 6:
                    nc.gpsimd.memset(vt[:], 0.0)
                pv = [psmm.tile([128, CH], f32, tag="mm", name=f"pv{tt}_{i}")
                      for i in range(2)]
                for kc in range(8):
                    if tt == 6:
                        xsl = xab[kc][:, 768:772]
                    else:
                        xsl = xab[kc][:, tt * 128:(tt + 1) * 128]
                    for dh in range(2):
                        nc.tensor.matmul(
                            pv[dh][:, 0:CH] if tt != 6 else pv[dh][0:4, 0:CH],
                            xsl, wv_t[kc][:, dh * 512:(dh + 1) * 512],
                            start=(kc == 0), stop=(kc == 7),
                        )
                nrow = 128 if tt != 6 else 4
                for dh in range(2):
                    nc.scalar.copy(
                        vr[0:nrow, dh * 8:(dh + 1) * 8, 0:64],
                        pv[dh][0:nrow].rearrange("p (h e) -> p h e", e=64),
                    )
                nc.scalar.copy(vr[:, :, 64:65],
                               tones[:].rearrange("p (h o) -> p h o", o=1))
                v_sb.append(vt)

            # ---------- attention blocks ----------
            def sc_half(hp, half, qT, kT):
                dsl = slice(half * 64, half * 64 + 64)
                sts = []
                for tidx in range(2):
                    st = pssc.tile([128, 1024], f32, tag="sc",
                                   name=f"st{hp}_{half}_{tidx}")
                    for (ti, coff, kc, w, qoff) in CHUNKS:
                        if ti != tidx:
                            continue
                        nc.tensor.matmul(
                            st[:, coff:coff + w],
                            kT[dsl, kc * 128:(kc + 1) * 128],
                            qT[dsl, qoff:qoff + w], start=True, stop=True,
                        )
                    sts.append(st)
                return sts

            def exp_mask(hp, half, sts):
                pts = []
                for tidx, st in enumerate(sts):
                    praw = ptp.tile([128, 1024], bf16, tag="pt",
                                    name=f"praw{hp}_{half}_{tidx}")
                    nc.scalar.activation(praw[:], st[:], AF.Exp, scale=0.125)
                    pt = ptp.tile([128, 1024], bf16, tag="pt",
                                  name=f"pt{hp}_{half}_{tidx}")
                    nc.vector.tensor_mul(
                        pt[:], praw[:],
                        tmask[:, tidx * 1024:(tidx + 1) * 1024])
                    pts.append(pt)
                return pts

            # AV order: sink chunk first — its full 512-col window
            # initializes every yt element (start=True), the rest accumulate.
            AV_CHUNKS = [CHUNKS[6]] + CHUNKS[0:6]

            def av_half(hp, half, pts):
                h = hp * 2 + half
                yt = psmm.tile([128, CH], f32, tag="mm",
                               name=f"yt{hp}_{half}")
                for ci, (ti, coff, kc, w, qoff) in enumerate(AV_CHUNKS):
                    nc.tensor.matmul(
                        yt[0:65, qoff:qoff + w],
                        v_sb[kc][:, h * 65:(h + 1) * 65],
                        pts[ti][:, coff:coff + w],
                        start=(ci == 0), stop=(ci == 6),
                        skip_group_check=True,
                    )
                # denominator -> 1/denominator (bf16) while still hot
                dt_ = tmp.tile([1, CH], f32, tag="dt", name=f"dt{hp}_{half}")
                nc.scalar.copy(dt_[:], yt[64:65, :])
                rf = tmp.tile([1, CH], f32, tag="rf", name=f"rf{hp}_{half}")
                nc.vector.reciprocal_approx_fast(rf[:], dt_[:])
                rb = tmp.tile([1, CH], bf16, tag="rb", name=f"rb{hp}_{half}")
                nc.vector.tensor_copy(rb[:], rf[:])
                nc.scalar.copy(ytu[hp][half * 64:half * 64 + 64, :],
                               yt[0:64, :])
                return rb

            def norm_hp(hp, rb0, rb1):
                # broadcast each half's 1/denom down 64 partitions via a
                # rank-1 matmul (ones row from sel2), then one fused multiply
                prb = psmm.tile([128, CH], f32, tag="mm", name=f"prb{hp}")
                nc.tensor.matmul(prb[0:64, :], tsel2[0:1, 0:64], rb0[:],
                                 start=True, stop=True, skip_group_check=True)
                nc.tensor.matmul(prb[64:128, :], tsel2[0:1, 0:64], rb1[:],
                                 start=True, stop=True, skip_group_check=True)
                nc.vector.tensor_mul(ytu[hp][:], ytu[hp][:], prb[:])

            # ---------- software-pipelined head-pair loop ----------
            qT, kT = qT0, kT0
            prev_norm = None
            for hp in range(8):
                sts0 = sc_half(hp, 0, qT, kT)
                pts0 = exp_mask(hp, 0, sts0)
                nqT = rope_q(hp + 1) if hp < 7 else None
                if prev_norm is not None:
                    norm_hp(*prev_norm)
                sts1 = sc_half(hp, 1, qT, kT)
                pts1 = exp_mask(hp, 1, sts1)
                nkT = rope_k(hp + 1) if hp < 7 else None
                rb0 = av_half(hp, 0, pts0)
                rb1 = av_half(hp, 1, pts1)
                prev_norm = (hp, rb0, rb1)
                qT, kT = nqT, nkT
            norm_hp(*prev_norm)

            # ---------- projection (packed transposed output) ----------
            osball = big.tile([128, 8 * CH], bf16, tag="osball")
            for cc in range(8):
                po = psmm.tile([128, CH], f32, tag="mm", name=f"po{cc}")
                for hp in range(8):
                    nc.tensor.matmul(
                        po[:], wp_t[cc][:, hp * 128:(hp + 1) * 128],
                        ytu[hp][:], start=(hp == 0), stop=(hp == 7),
                    )
                nc.scalar.copy(osball[:, cc * CH:(cc + 1) * CH], po[:])
                if cc == 3:
                    nc.sync.dma_start(outT[:, 0:4 * CH],
                                      osball[:, 0:4 * CH])
            nc.sync.dma_start(outT[:, 4 * CH:8 * CH],
                              osball[:, 4 * CH:8 * CH])

    nc.compile()
    return nc


def _host_inputs(x, w_attn, w_proj):
    """Build the 8 per-core input maps (bf16 operands)."""
    import ml_dtypes
    bf = ml_dtypes.bfloat16

    inv_freq = 1.0 / (10000.0 ** (np.arange(0, HD, 2, dtype=np.float32) / HD))
    iff = np.concatenate([inv_freq, inv_freq])  # [64]

    def cos_sin(pos):
        ang = pos[None, :].astype(np.float32) * iff[:, None]
        c = np.concatenate([np.cos(ang), np.cos(ang)], 0)
        s = np.concatenate([np.sin(ang), np.sin(ang)], 0)
        return (np.ascontiguousarray(c.astype(bf)),
                np.ascontiguousarray(s.astype(bf)))

    P2 = np.zeros((128, 128), np.float32)
    for blk in range(2):
        o = blk * 64
        for d in range(32):
            P2[o + d + 32, o + d] = -1.0
            P2[o + d, o + d + 32] = 1.0

    sel2 = np.zeros((2, 128), np.float32)
    sel2[0, 0:64] = 1.0
    sel2[1, 64:128] = 1.0
    ones16 = np.ones((128, 16), np.float32)

    def shuffle_lhsT(w):
        # rows (kc*128 + c_lo), cols (hp*128 + d) ->
        # rows (hp*128 + c_lo), cols (kc*128 + d), then packed so SBUF
        # partition p carries all 8 hp-blocks contiguously (16KB DMA lines)
        sh = w.reshape(8, 128, 8, 128).transpose(2, 1, 0, 3).reshape(C, C)
        return np.ascontiguousarray(
            sh.reshape(8, 128, C).transpose(1, 0, 2).reshape(128, 8 * C))

    def pack_rows(w):
        # [8*128, N] -> [128, 8*N]: partition p holds all 8 row-blocks
        n = w.shape[1]
        return np.ascontiguousarray(
            w.reshape(8, 128, n).transpose(1, 0, 2).reshape(128, 8 * n))

    wq = shuffle_lhsT(w_attn[:, 0:C]).astype(bf)
    wk = shuffle_lhsT(w_attn[:, C:2 * C]).astype(bf)
    wvm = pack_rows(np.ascontiguousarray(w_attn[:, 2 * C:3 * C])).astype(bf)
    wp = shuffle_lhsT(w_proj).astype(bf)
    P2 = P2.astype(bf)
    sel2 = sel2.astype(bf)
    ones16 = ones16.astype(bf)

    in_maps = []
    for core in range(NCORES):
        b, j = core // 4, core % 4
        q0 = j * CH
        # kv layout: [512 own | 256 halo | 4 sink]
        kv_gk = np.full(KV, -1, np.int64)
        kv_gk[0:512] = q0 + np.arange(CH)
        halo = q0 - 256 + np.arange(256)
        kv_gk[512:768] = np.where(halo >= 0, halo, -1)
        kv_gk[768:772] = np.arange(4)

        xTc = np.zeros((C, KV), np.float32)
        valid = kv_gk >= 0
        xTc[:, valid] = x[b, kv_gk[valid]].T

        cq, sq = cos_sin(q0 + np.arange(CH))
        ck, sk = cos_sin(np.maximum(kv_gk, 0))

        mask = np.zeros((128, MTOT), np.float32)
        for (tidx, coff, kc, w, qoff) in CHUNKS:
            p = np.arange(128)
            if kc < 4:
                g = q0 + kc * 128 + p
                real = np.ones(128, bool)
            elif kc in (4, 5):
                g = q0 - 256 + (kc - 4) * 128 + p
                real = g >= 0
            else:
                g = p.copy()
                real = p < 4
            gq = q0 + qoff + np.arange(w)
            gcol = np.where(real, g, 0)[:, None]
            qq = gq[None, :]
            if kc == 6:
                allow = (gcol <= qq) & (qq - gcol >= WIN)
            else:
                allow = (gcol <= qq) & (qq - gcol < WIN)
            allow &= real[:, None]
            mask[:, tidx * 1024 + coff: tidx * 1024 + coff + w] = \
                allow.astype(np.float32)

        in_maps.append({
            "xT": pack_rows(xTc).astype(bf), "wqs": wq, "wks": wk,
            "wv": wvm, "wps": wp,
            "cos_q": cq, "sin_q": sq, "cos_k": ck, "sin_k": sk,
            "masks": mask.astype(bf), "p2": P2, "sel2": sel2, "ones": ones16,
        })
    return in_maps


def kernel(x, w_attn, w_proj):
    from concourse import bass_utils

    x = np.asarray(x, np.float32)
    w_attn = np.asarray(w_attn, np.float32)
    w_proj = np.asarray(w_proj, np.float32)

    if "nc" not in _cache:
        _cache["nc"] = _build_nc()
    nc = _cache["nc"]

    in_maps = _host_inputs(x, w_attn, w_proj)
    res = bass_utils.run_bass_kernel_spmd(nc, in_maps, list(range(NCORES)),
                                          **_cache.get("run_kwargs", {}))
    _cache["last_result"] = res

    y = np.zeros((B, T, C), np.float32)
    for core in range(NCORES):
        b, j = core // 4, core % 4
        o = np.asarray(res.results[core]["outT"], np.float32)
        y[b, j * CH:(j + 1) * CH, :] = \
            o.reshape(128, 8, CH).transpose(1, 0, 2).reshape(C, CH).T
    return y
